# revision 46
# baseline (speedup 1.0000x reference)
"""Multi-head attention (B=2, N=2048, C=1024, H=16, D=64) on 8 TRN2 NeuronCores.

Sharding: core = b*4 + g  (b: data parallel over batch, g: tensor parallel over
head groups of HL=4 heads).

Per-core pipeline (all matmuls bf16; attention processed per (head, n-half)):
  phase 0: QT/KT = w.T @ xT -> [DL, N] (bias added on DVE, + partition-swapped
           copies for A-phase row-group pairing); V = xT.T @ wvT -> [N, DL]
           (+1 ones column; v-bias folded in via a broadcast tile add)
  per head h, per n-half f (1024 cols), per m-chunk i (128 tokens):
    A: S^T chunk = KT_h[:,i].T @ QT_h[:,f] -> psum [128, 1024]; even/odd chunks
       use PE row groups 0-63 / 64-127 so consecutive A matmuls overlap
    exp: E = exp(S^T) -> bf16 tile; split between ScalarE (exact) and
       VectorE (bf16 Schraudolph bit trick) to raise exp throughput
    B: O^T[65, 1024] += V_aug[:,i,h].T @ E  (row 64 = softmax denominator)
  norm: rc = 1/den (DVE, from psum); bc = ones.T @ rc (PE broadcast, staged to
       SBUF on ScalarE); yt = O^T * bc (DVE)
  phase D: P^T = pw.T @ Y^T -> [C, N] partial projection, staged + DMA'd out
Phase-0 and phase-D matmuls are interleaved into the attention stream as PE
filler so the tensor engine keeps streaming while the exps catch up.

Host: out[b] = sum_g P^T[b,g].T + proj_b
"""

import numpy as np
import ml_dtypes
from collections import deque

B, N, C = 2, 2048, 1024
H = 16
D = C // H          # 64
G = 4               # head groups (tensor parallel)
HL = H // G         # 4 heads per core
DL = HL * D         # 256 local head dims
N_CORES = 8
SCALE = 1.0 / np.sqrt(np.float32(D))

MCHUNKS = N // 128  # 16
MO = DL // 128      # 2
CO = C // 128       # 8
NF = 1024           # n-half width

SCH_A = 128.0 / np.log(2.0)                 # bf16 Schraudolph scale
SCH_B = 16256.0 - 128.0 * 0.0430 - 1.874    # offset, bias-recentered
EXP_OFFLOAD = True    # route 1/4 of exps to DVE (Schraudolph bf16 bit trick)
A_MODE = "pair"       # "pair": even/odd chunks on PE row groups 0/64; "nopair0": all at 0
QK_F32R = False       # Q/K tiles f32r instead of bf16 (walrus bf16+rowtile workaround)

_CACHE = {}


def build_kernel():
    import concourse.mybir as mybir
    import concourse.tile as tile
    from concourse import bacc

    f32 = mybir.dt.float32
    f32r = mybir.dt.float32r
    bf16 = mybir.dt.bfloat16
    i16 = mybir.dt.int16
    EXP = mybir.ActivationFunctionType.Exp
    MUL = mybir.AluOpType.mult

    nc = bacc.Bacc("TRN2", target_bir_lowering=False, debug=False,
                   num_devices=N_CORES)

    xt_d = nc.dram_tensor("xt", [C, N], bf16, kind="ExternalInput").ap()
    wqt_d = nc.dram_tensor("wqt", [C, DL], bf16, kind="ExternalInput").ap()
    wkt_d = nc.dram_tensor("wkt", [C, DL], bf16, kind="ExternalInput").ap()
    wvt_d = nc.dram_tensor("wvt", [C, DL], bf16, kind="ExternalInput").ap()
    bq_d = nc.dram_tensor("bq", [128, MO], f32, kind="ExternalInput").ap()
    bk_d = nc.dram_tensor("bk", [128, MO], f32, kind="ExternalInput").ap()
    bv_d = nc.dram_tensor("bv", [1, DL], f32, kind="ExternalInput").ap()
    pwt_d = nc.dram_tensor("pwt", [DL, C], bf16, kind="ExternalInput").ap()
    f16 = mybir.dt.float16
    out_d = nc.dram_tensor("out", [C, N], f16, kind="ExternalOutput").ap()

    with tile.TileContext(nc) as tc:
        with (
            tc.tile_pool(name="consts", bufs=1) as consts,
            tc.tile_pool(name="acts", bufs=1) as acts,
            tc.tile_pool(name="small", bufs=2) as small,
            tc.tile_pool(name="eip", bufs=14) as eip,
            tc.tile_pool(name="stg", bufs=3) as stg,
            tc.tile_pool(name="psS", bufs=3, space="PSUM") as psS,
            tc.tile_pool(name="psO", bufs=1, space="PSUM") as psO,
        ):
            # ---- weights / consts ----
            # DMA order matters: the K/Q phase-0 matmuls need bias, wq/wk and
            # early xt chunks first; wv and pw are consumed much later.
            bq_sb = consts.tile([128, MO], f32, tag="bq")
            bk_sb = consts.tile([128, MO], f32, tag="bk")
            nc.sync.dma_start(bq_sb[:], bq_d[:])
            nc.sync.dma_start(bk_sb[:], bk_d[:])
            bv_sb = consts.tile([1, DL], f32, tag="bv")
            nc.sync.dma_start(bv_sb[:], bv_d[:])
            wq_sb = consts.tile([128, CO, DL], bf16, tag="wq")
            wk_sb = consts.tile([128, CO, DL], bf16, tag="wk")
            wv_sb = consts.tile([128, CO, DL], bf16, tag="wv")
            xt_sb = acts.tile([128, CO, N], bf16, tag="xt")
            xt_r = xt_d.rearrange("(o p) n -> p o n", p=128)
            wq_r = wqt_d.rearrange("(o p) f -> p o f", p=128)
            wk_r = wkt_d.rearrange("(o p) f -> p o f", p=128)
            # per-chunk DMAs so the first K/Q matmuls only wait on one chunk
            for kc in range(CO):
                nc.sync.dma_start(xt_sb[:, kc, :], xt_r[:, kc, :])
                nc.sync.dma_start(wk_sb[:, kc, :], wk_r[:, kc, :])
                nc.sync.dma_start(wq_sb[:, kc, :], wq_r[:, kc, :])
            nc.sync.dma_start(wv_sb[:], wvt_d.rearrange("(o p) f -> p o f", p=128))
            pw_sb = consts.tile([128, MO, C], bf16, tag="pw")
            nc.sync.dma_start(pw_sb[:], pwt_d.rearrange("(o p) f -> p o f", p=128))
            ones_col = consts.tile([128, 1], f32, tag="onescol")
            nc.vector.memset(ones_col[:], 1.0)

            # ---- PE warmup: ~5us of dependency-free matmuls during the DMA
            # wait, so the HAM activity window fires and the kernel starts at
            # the warm 2.4 GHz clock instead of the cold 1.2 GHz default ----
            warm_sb = consts.tile([128, 512], bf16, tag="warm")
            nc.vector.memset(warm_sb[:], 0.125)
            for r in range(56):
                wps = psS.tile([128, NF], f32, tag="ps", name=f"warm_{r}")
                nc.tensor.matmul(wps[:, 0:512], lhsT=warm_sb[:, 0:128],
                                 rhs=warm_sb[:], start=True, stop=True)

            # ---- resident activations ----
            qk_dt = f32r if QK_F32R else bf16
            qt_sb = acts.tile([128, MO, N], qk_dt, tag="qt")
            kt_sb = acts.tile([128, MO, N], qk_dt, tag="kt")
            qt2_sb = acts.tile([128, MO, N], qk_dt, tag="qt2")  # halves swapped
            kt2_sb = acts.tile([128, MO, N], qk_dt, tag="kt2")
            v_sb = acts.tile([128, MCHUNKS, HL, D + 1], bf16, tag="v")
            yt_sb = acts.tile([128, MO, N], bf16, tag="yt")
            bvb_sb = acts.tile([128, HL, D], f32, tag="bvb")

            nc.vector.tensor_copy(
                v_sb[:, :, :, D:],
                ones_col[:].to_broadcast([128, MCHUNKS, HL, 1]))

            # v-bias broadcast tile via GpSimd partition broadcast
            nc.gpsimd.partition_broadcast(bvb_sb[:], bv_sb[:])

            # ---------- phase-0 emitters ----------
            _uid = [0]

            def emit_qk(w_sb, b_sb, o_sb, o2_sb, mo, nh, halves=(0, 1),
                        beat=False):
                _uid[0] += 1
                aux = psS.tile([128, NF], f32, tag="ps",
                               name=f"qk_{_uid[0]}_{mo}_{nh}")
                for half in halves:
                    sl = slice(half * 512, half * 512 + 512)
                    for kc in range(CO):
                        if beat and half == 0 and kc % 2 == 0:
                            # fill xt-DMA waits with activity; writes the
                            # half-1 region, re-cleared by its first matmul
                            nc.tensor.matmul(
                                aux[:, 512:1024], lhsT=warm_sb[:, 0:128],
                                rhs=warm_sb[:], start=True, stop=True,
                                skip_group_check=True)
                        nc.tensor.matmul(
                            aux[:, sl],
                            lhsT=w_sb[:, kc, mo * 128:(mo + 1) * 128],
                            rhs=xt_sb[:, kc, nh * NF + half * 512:
                                      nh * NF + half * 512 + 512],
                            start=(kc == 0), stop=(kc == CO - 1),
                        )
                    nsl = slice(nh * NF + half * 512,
                                nh * NF + half * 512 + 512)
                    nc.vector.tensor_scalar_add(
                        o_sb[:, mo, nsl], aux[:, sl], b_sb[:, mo:mo + 1])
                    nc.vector.tensor_copy(o2_sb[0:64, mo, nsl],
                                          o_sb[64:128, mo, nsl])
                    nc.vector.tensor_copy(o2_sb[64:128, mo, nsl],
                                          o_sb[0:64, mo, nsl])

            def emit_v(i):
                aux = psS.tile([128, NF], f32, tag="ps", name=f"v_{i}")
                for kc in range(CO):
                    nc.tensor.matmul(
                        aux[:, 0:DL],
                        lhsT=xt_sb[:, kc, i * 128:(i + 1) * 128],
                        rhs=wv_sb[:, kc, :],
                        start=(kc == 0), stop=(kc == CO - 1),
                    )
                for hh in range(HL):
                    nc.vector.tensor_add(
                        v_sb[:, i, hh, :D],
                        aux[:, hh * D:(hh + 1) * D], bvb_sb[:, hh, :])

            def emit_b(h, o_ps, i, ei):
                for half in range(2):
                    sl = slice(half * 512, half * 512 + 512)
                    nc.tensor.matmul(
                        o_ps[:, sl],
                        lhsT=v_sb[:, i, h, :],
                        rhs=ei[:, sl],
                        start=(i == 0), stop=(i == MCHUNKS - 1),
                    )

            def emit_d(f, cc, fine=False):
                aux = psS.tile([128, NF], f32, tag="ps", name=f"d_{f}_{cc}")
                st = stg.tile([128, NF], f16, tag="st", name=f"st_{f}_{cc}")
                for half in range(2):
                    sl = slice(half * 512, half * 512 + 512)
                    nsl = slice(f * NF + half * 512, f * NF + half * 512 + 512)
                    for jc in range(MO):
                        nc.tensor.matmul(
                            aux[:, sl],
                            lhsT=pw_sb[:, jc, cc * 128:(cc + 1) * 128],
                            rhs=yt_sb[:, jc, nsl],
                            start=(jc == 0), stop=(jc == MO - 1),
                        )
                    if fine:
                        if (cc + half) % 2 == 0:
                            nc.vector.tensor_copy(st[:, sl], aux[:, sl])
                        else:
                            nc.scalar.copy(st[:, sl], aux[:, sl])
                        nc.sync.dma_start(
                            out_d[cc * 128:(cc + 1) * 128, nsl], st[:, sl])
                if not fine:
                    if cc % 2 == 0:
                        nc.vector.tensor_copy(st[:], aux[:])
                    else:
                        nc.scalar.copy(st[:], aux[:])
                    nc.sync.dma_start(
                        out_d[cc * 128:(cc + 1) * 128, f * NF:(f + 1) * NF],
                        st[:])

            def norm(h, f, o_ps):
                mo = h // 2
                pb = 64 * (h % 2)
                dn = small.tile([1, NF], f32, tag="dn", name=f"dn_{h}_{f}")
                rc = small.tile([1, NF], f32, tag="rc", name=f"rc_{h}_{f}")
                bcs = small.tile([64, NF], f32, tag="bcs", name=f"bcs_{h}_{f}")
                parts = ((slice(0, 512), slice(512, 1024))
                         if (h, f) == (HL - 1, 1) else (slice(0, NF),))
                for sl in parts:
                    nc.vector.tensor_copy(dn[:, sl], o_ps[D:D + 1, sl])
                    nc.vector.reciprocal_approx_fast(rc[:, sl], dn[:, sl])
                    nc.gpsimd.partition_broadcast(bcs[:, sl], rc[:, sl])
                    nc.vector.tensor_mul(
                        yt_sb[pb:pb + D, mo,
                              f * NF + sl.start:f * NF + sl.stop],
                        o_ps[0:D, sl], bcs[:, sl])

            # ---------- filler queue (phase-0 remainder) ----------
            filler_q = deque()
            for nh in range(2):
                filler_q.append(
                    (lambda nh_: lambda: emit_qk(
                        wk_sb, bk_sb, kt_sb, kt2_sb, 1, nh_))(nh))
            for nh in range(2):
                filler_q.append(
                    (lambda nh_: lambda: emit_qk(
                        wq_sb, bq_sb, qt_sb, qt2_sb, 1, nh_))(nh))

            def pop_fillers(k):
                for _ in range(k):
                    if not filler_q:
                        return
                    filler_q.popleft()()

            # ---------- prologue ----------
            emit_qk(wk_sb, bk_sb, kt_sb, kt2_sb, 0, 0, beat=True)
            emit_qk(wq_sb, bq_sb, qt_sb, qt2_sb, 0, 0, beat=True)
            emit_v(0)

            # ---------- attention: one flat software-pipelined stream ----
            # B matmuls trail their A/exp by LAG blocks; a pass's B-tail and
            # norm are emitted inside the next pass's early blocks so the PE
            # queue never waits on an exp or the norm's DVE chain.
            LAG = 4
            v_next = 1
            pending = deque()      # (t_emitted, h, f, i, ei, o_ps)
            o_tiles = {}
            b_count = {}

            def pop_b(h_, f_, i_, ei_):
                if (h_, f_) not in o_tiles:
                    o_tiles[(h_, f_)] = psO.tile(
                        [D + 1, NF], f32, tag="po", name=f"o_{h_}_{f_}")
                    b_count[(h_, f_)] = 0
                o_ps = o_tiles[(h_, f_)]
                emit_b(h_, o_ps, i_, ei_)
                b_count[(h_, f_)] += 1
                if b_count[(h_, f_)] == MCHUNKS:
                    norm(h_, f_, o_ps)
                    del o_tiles[(h_, f_)]
                    if h_ == HL - 1:
                        # phase D for this n-half: f0 as pass-7 fillers,
                        # f1 emitted at the tail
                        for cc in range(CO):
                            if f_ == 0:
                                filler_q.append(
                                    (lambda cc_: lambda: emit_d(
                                        0, cc_, fine=True))(cc))
                            else:
                                emit_d(1, cc, fine=True)

            t = 0
            for pass_idx, (h, f) in enumerate(
                    (h, f) for h in range(HL) for f in range(2)):
                mo = h // 2
                p_lo = 64 * (h % 2)
                p_hi = p_lo ^ 64
                for ib in range(MCHUNKS // 2):
                    ie, io = 2 * ib, 2 * ib + 1
                    pse = psS.tile([128, NF], f32, tag="ps",
                                   name=f"sA_{h}_{f}_{ie}")
                    pso = psS.tile([128, NF], f32, tag="ps",
                                   name=f"sA_{h}_{f}_{io}")
                    if pass_idx >= 2:
                        # HAM heartbeat: dependency-free matmul into the S
                        # tile; the A matmul's start=True overwrites it. Keeps
                        # the PE activity window busy so the clock stays warm.
                        nc.tensor.matmul(pse[:, 0:512], lhsT=warm_sb[:, 0:128],
                                         rhs=warm_sb[:], start=True, stop=True,
                                         skip_group_check=True)
                    if A_MODE == "pair":
                        parts = (
                            (kt_sb, qt_sb, p_lo, ie, pse),
                            (kt2_sb, qt2_sb, p_hi, io, pso),
                        )
                    else:
                        kts, qts = ((kt_sb, qt_sb) if h % 2 == 0
                                    else (kt2_sb, qt2_sb))
                        parts = (
                            (kts, qts, 0, ie, pse),
                            (kts, qts, 0, io, pso),
                        )
                    for half in range(2):
                        sl = slice(half * 512, half * 512 + 512)
                        fsl = slice(f * NF + half * 512,
                                    f * NF + half * 512 + 512)
                        for kts_, qts_, pb_, ii_, ps_ in parts:
                            nc.tensor.matmul(
                                ps_[:, sl],
                                lhsT=kts_[pb_:pb_ + D, mo,
                                          ii_ * 128:(ii_ + 1) * 128],
                                rhs=qts_[pb_:pb_ + D, mo, fsl],
                                start=True, stop=True)
                    for i, ps in ((ie, pse), (io, pso)):
                        ei = eip.tile([128, NF], bf16, tag="ei",
                                      name=f"ei_{h}_{f}_{i}")
                        if EXP_OFFLOAD and pass_idx >= 1 and i % 4 == 3:
                            nc.vector.tensor_scalar(
                                ei.bitcast(i16)[:], ps[:], SCH_A, SCH_B,
                                MUL, mybir.AluOpType.add)
                        else:
                            nc.scalar.activation(ei[:], ps[:], EXP)
                        pending.append((t, h, f, i, ei))
                    # PE fillers
                    if pass_idx == 0:
                        if t == 1:
                            emit_qk(wk_sb, bk_sb, kt_sb, kt2_sb, 0, 1)
                        else:
                            for _ in range(3 if t in (2, 3) else 2):
                                if v_next < MCHUNKS:
                                    emit_v(v_next)
                                    v_next += 1
                    elif filler_q and pass_idx == 7:
                        if ib >= 1:
                            pop_fillers(1 if ib < 6 else 2)
                    elif filler_q and ib % 2 == 1:
                        pop_fillers(1)
                    # trailing B's; a pass's first B (start=True, reuses the
                    # single O psum buffer) is held until in-pass block 4 so
                    # the previous norm's chain has finished reading it
                    while pending and pending[0][0] <= t - LAG:
                        if pending[0][3] == 0 and t % 8 < 5:
                            break
                        _, h_, f_, i_, ei_ = pending.popleft()
                        pop_b(h_, f_, i_, ei_)
                    if pass_idx == 0 and ib == MCHUNKS // 2 - 1:
                        # Q(mo0, f1) before pass 1's A matmuls read it
                        emit_qk(wq_sb, bq_sb, qt_sb, qt2_sb, 0, 1)
                    t += 1
            while pending:
                _, h_, f_, i_, ei_ = pending.popleft()
                pop_b(h_, f_, i_, ei_)
            pop_fillers(99)

    nc.compile()
    return nc


def shard_inputs(x, qkv_w, qkv_b, proj_w):
    in_maps = []
    for core in range(N_CORES):
        b, g = divmod(core, G)
        gs = slice(g * DL, (g + 1) * DL)
        xt = np.ascontiguousarray(x[b].T)
        wq = qkv_w[0 * C:1 * C][gs] * SCALE
        wk = qkv_w[1 * C:2 * C][gs]
        wv = qkv_w[2 * C:3 * C][gs]
        in_maps.append({
            "xt": xt.astype(ml_dtypes.bfloat16),
            "wqt": np.ascontiguousarray(wq.T).astype(ml_dtypes.bfloat16),
            "wkt": np.ascontiguousarray(wk.T).astype(ml_dtypes.bfloat16),
            "wvt": np.ascontiguousarray(wv.T).astype(ml_dtypes.bfloat16),
            "bq": np.ascontiguousarray(
                (qkv_b[0 * C:1 * C][gs] * SCALE).reshape(MO, 128).T),
            "bk": np.ascontiguousarray(
                qkv_b[1 * C:2 * C][gs].reshape(MO, 128).T),
            "bv": np.ascontiguousarray(qkv_b[2 * C:3 * C][gs].reshape(1, DL)),
            "pwt": np.ascontiguousarray(proj_w[:, gs].T).astype(
                ml_dtypes.bfloat16),
        })
    return in_maps


def unshard_output(results, proj_b):
    out = np.empty((B, N, C), dtype=np.float32)
    for b in range(B):
        acc = results[b * G]["out"].astype(np.float32)
        for g in range(1, G):
            acc = acc + results[b * G + g]["out"]
        out[b] = acc.T + proj_b
    return out


def kernel(x, qkv_w, qkv_b, proj_w, proj_b):
    from concourse.bass_utils import run_bass_kernel_spmd

    x = np.asarray(x, dtype=np.float32)
    qkv_w = np.asarray(qkv_w, dtype=np.float32)
    qkv_b = np.asarray(qkv_b, dtype=np.float32)
    proj_w = np.asarray(proj_w, dtype=np.float32)
    proj_b = np.asarray(proj_b, dtype=np.float32)

    if "nc" not in _CACHE:
        _CACHE["nc"] = build_kernel()
    nc = _CACHE["nc"]

    in_maps = shard_inputs(x, qkv_w, qkv_b, proj_w)
    res = run_bass_kernel_spmd(nc, in_maps, list(range(N_CORES)))
    return unshard_output(res.results, proj_b)
